# revision 6
# baseline (speedup 1.0000x reference)
"""Trainium2 Bass kernel for nn_LlamaAttention_6588479832091.

Math notes:
  - The reference attention contracts q and k at the SAME sequence position
    (scores = einsum('bshd,bstd->bsht', q, k)), and RoPE applies the same
    orthogonal transform to q and k at equal positions, so RoPE cancels
    exactly: (P R q)·(P R k) = q·k.  v and the output path never see RoPE.
    The kernel therefore computes: q/k/v projections, per-token 16x16
    cross-head softmax attention, and the output projection.
  - Sharding: data-parallel over the 16384 tokens -> 2048 tokens per core,
    weights replicated.  No collectives.
  - Projections run as float32r matmuls (full PE speed, ~1.4e-4 rel err);
    the small attention matmuls run in plain fp32.

Layouts (host-prepared, all "partition-first" 3D):
  xt   [128, 16, 2048]  xt[p, ct, t] = x_shard[t, ct*128+p]           (f32r)
  wqt  [128, 16, 2048]  wqt[p, ct, m] = wq[m, ct*128+p] / sqrt(128)   (f32r)
  wkt, wvt: same layout as wqt (wk, wv unscaled)                      (f32r)
  wot  [128, 16, 2048]  wot[p, mt, r] = wo[r, mt*128+p]               (f32r)
  mask [128, 128]       0 where p%8 == n%8 else -30000                (f32)
  ident[128, 128]       identity                                      (f32)
  ot   [128, 16, 2048]  ot[p, rt, t] = out_shard[t, rt*128+p]         (f32, output)
"""
import sys

for _p in ("/opt/trn_rl_repo", "/root/.axon_site/_ro/trn_rl_repo"):
    if _p not in sys.path:
        sys.path.insert(0, _p)

import numpy as np

T_CORE = 2048      # tokens per core
N_CORES = 8
H = 16             # heads
HD = 128           # head dim
HIDDEN = 2048
CT = HIDDEN // 128  # 16 contraction tiles
TCH = 512          # token chunk for N=512 matmuls
SUB = 256          # attention sub-chunk tokens
GRP = 8            # tokens per attention group

_CACHED = {}


def _build():
    import concourse.mybir as mybir
    import concourse.tile as tile
    import concourse.bacc as bacc

    f32 = mybir.dt.float32
    f32r = mybir.dt.float32r

    nc = bacc.Bacc("TRN2", target_bir_lowering=False, debug=False)

    xt = nc.declare_dram_parameter("xt", [128, CT, T_CORE], f32r, isOutput=False)
    wqt = nc.declare_dram_parameter("wqt", [128, CT, HIDDEN], f32r, isOutput=False)
    wkt = nc.declare_dram_parameter("wkt", [128, CT, HIDDEN], f32r, isOutput=False)
    wvt = nc.declare_dram_parameter("wvt", [128, CT, HIDDEN], f32r, isOutput=False)
    wot = nc.declare_dram_parameter("wot", [128, CT, HIDDEN], f32r, isOutput=False)
    maskd = nc.declare_dram_parameter("maskd", [128, 128], f32, isOutput=False)
    identd = nc.declare_dram_parameter("identd", [128, 128], f32, isOutput=False)
    ot = nc.declare_dram_parameter("ot", [128, CT, T_CORE], f32, isOutput=True)

    with tile.TileContext(nc) as tc:
        with tc.tile_pool(name="dram", bufs=1, space="DRAM") as dram:
            qT = dram.tile([128, H, T_CORE], f32, name="qT")
            kT = dram.tile([128, H, T_CORE], f32, name="kT")
            vT = dram.tile([128, H, T_CORE], f32, name="vT")
            aT = dram.tile([128, H, T_CORE], f32, name="aT")

            # ---------------- Phase P: q/k/v projections (f32r) -------------
            with tc.tile_pool(name="p_x", bufs=1) as p_x, \
                 tc.tile_pool(name="p_w", bufs=3) as p_w, \
                 tc.tile_pool(name="p_ev", bufs=4) as p_ev, \
                 tc.tile_pool(name="p_ps", bufs=4, space="PSUM") as p_ps:
                x_sb = p_x.tile([128, CT, T_CORE], f32r)
                nc.sync.dma_start(x_sb[:], xt[:])
                for wsrc, qdst in ((wqt, qT), (wkt, kT), (wvt, vT)):
                    for mt in range(H):
                        wslab = p_w.tile([128, CT, 128], f32r, tag="wslab")
                        nc.sync.dma_start(wslab[:], wsrc[:, :, mt * 128:(mt + 1) * 128])
                        for tch in range(T_CORE // TCH):
                            psum = p_ps.tile([128, TCH], f32, tag="pp")
                            for kt in range(CT):
                                nc.tensor.matmul(
                                    psum[:],
                                    wslab[:, kt, :],
                                    x_sb[:, kt, tch * TCH:(tch + 1) * TCH],
                                    start=(kt == 0), stop=(kt == CT - 1))
                            ev = p_ev.tile([128, TCH], f32, tag="ev")
                            nc.vector.tensor_copy(ev[:], psum[:])
                            nc.sync.dma_start(
                                qdst[:, mt, tch * TCH:(tch + 1) * TCH], ev[:])

            # ---------------- Phase A: cross-head attention ------------------
            with tc.tile_pool(name="a_io", bufs=1) as a_io, \
                 tc.tile_pool(name="a_wk", bufs=3) as a_wk, \
                 tc.tile_pool(name="a_ps", bufs=1, space="PSUM") as a_ps:
                mask_sb = a_io.tile([128, 128], f32, tag="mask")
                ident_sb = a_io.tile([128, 128], f32, tag="ident")
                nc.sync.dma_start(mask_sb[:], maskd[:])
                nc.sync.dma_start(ident_sb[:], identd[:])
                NG = SUB // GRP
                for sub in range(T_CORE // SUB):
                    sl = slice(sub * SUB, (sub + 1) * SUB)
                    # load q/k/v directly in group-packed layout:
                    # pk[d, g, h*8+tj] = T[d, h, sub*SUB + g*8 + tj]
                    q_pk = a_io.tile([128, NG, 128], f32, tag="q", bufs=2)
                    k_pk = a_io.tile([128, NG, 128], f32, tag="k", bufs=2)
                    v_pk = a_io.tile([128, NG, 128], f32, tag="v", bufs=2)
                    at_sb = a_io.tile([128, H, SUB], f32, tag="at", bufs=2)
                    for pk, srcd in ((q_pk, qT), (k_pk, kT), (v_pk, vT)):
                        for h in range(H):
                            nc.sync.dma_start(
                                pk[:, :, h * GRP:(h + 1) * GRP],
                                srcd[:, h, sl].rearrange("p (g tj) -> p g tj", tj=GRP))
                    for g in range(NG):
                        t0 = g * GRP
                        qap = q_pk[:, g, :]   # cols (h, ti)
                        vap = v_pk[:, g, :]
                        ps_s = a_ps.tile([128, 128], f32, tag="s", bufs=2)
                        nc.tensor.matmul(ps_s[:], qap, k_pk[:, g, :],
                                         start=True, stop=True)
                        s_sb = a_wk.tile([128, 128], f32, tag="s_sb")
                        nc.vector.tensor_add(s_sb[:], ps_s[:], mask_sb[:])
                        w_sb = a_wk.tile([128, 128], f32, tag="w_sb")
                        zacc = a_wk.tile([128, 1], f32, tag="z")
                        nc.scalar.activation(w_sb[:], s_sb[:],
                                             mybir.ActivationFunctionType.Exp,
                                             accum_out=zacc[:])
                        rz = a_wk.tile([128, 1], f32, tag="rz")
                        nc.vector.reciprocal(rz[:], zacc[:])
                        ps_wt = a_ps.tile([128, 128], f32, tag="wt")
                        nc.tensor.transpose(ps_wt[:], w_sb[:], ident_sb[:])
                        wt_sb = a_wk.tile([128, 128], f32, tag="wt_sb")
                        nc.vector.tensor_copy(wt_sb[:], ps_wt[:])
                        ps_v = a_ps.tile([128, 128], f32, tag="vp")
                        nc.tensor.transpose(ps_v[:], vap, ident_sb[:])
                        vp_sb = a_wk.tile([128, 128], f32, tag="vp_sb")
                        nc.vector.tensor_copy(vp_sb[:], ps_v[:])
                        ps_at = a_ps.tile([128, 128], f32, tag="attn", bufs=2)
                        nc.tensor.matmul(ps_at[:], wt_sb[:], vp_sb[:],
                                         start=True, stop=True)
                        an_sb = a_wk.tile([128, 128], f32, tag="an_sb")
                        nc.vector.tensor_scalar_mul(an_sb[:], ps_at[:], rz[:])
                        ps_aT = a_ps.tile([128, 128], f32, tag="aTp")
                        nc.tensor.transpose(ps_aT[:], an_sb[:], ident_sb[:])
                        nc.vector.tensor_copy(at_sb[:, :, t0:t0 + GRP], ps_aT[:])
                    nc.sync.dma_start(aT[:, :, sl], at_sb[:])

            # ---------------- Phase O: output projection (f32r) --------------
            with tc.tile_pool(name="o_w", bufs=1) as o_w, \
                 tc.tile_pool(name="o_a", bufs=2) as o_a, \
                 tc.tile_pool(name="o_ev", bufs=4) as o_ev, \
                 tc.tile_pool(name="o_ps", bufs=4, space="PSUM") as o_ps:
                wo_sb = o_w.tile([128, CT, HIDDEN], f32r)
                nc.sync.dma_start(wo_sb[:], wot[:])
                for tch in range(T_CORE // TCH):
                    a_sb = o_a.tile([128, H, TCH], f32r, tag="a")
                    nc.gpsimd.dma_start(
                        a_sb[:], aT[:, :, tch * TCH:(tch + 1) * TCH])
                    for rt in range(CT):
                        psum = o_ps.tile([128, TCH], f32, tag="po")
                        for kt in range(CT):
                            nc.tensor.matmul(
                                psum[:],
                                wo_sb[:, kt, rt * 128:(rt + 1) * 128],
                                a_sb[:, kt, :],
                                start=(kt == 0), stop=(kt == CT - 1))
                        ev = o_ev.tile([128, TCH], f32, tag="oev")
                        nc.vector.tensor_copy(ev[:], psum[:])
                        nc.sync.dma_start(
                            ot[:, rt, tch * TCH:(tch + 1) * TCH], ev[:])
    nc.compile()
    return nc


def _host_prep(x, wq, wk, wv, wo):
    """Build per-core input maps (layout transforms only)."""
    x2 = np.ascontiguousarray(x.reshape(-1, HIDDEN))          # (16384, 2048)
    wqs = (wq / np.sqrt(np.float32(HD))).astype(np.float32)

    def wt3(w):   # (m, c) weight -> [128, CT, HIDDEN] with w.T tiled on c
        wt = np.ascontiguousarray(w.T)                        # (c, m)
        return np.ascontiguousarray(
            wt.reshape(CT, 128, HIDDEN).transpose(1, 0, 2))

    wqt, wkt, wvt, wot = wt3(wqs), wt3(wk), wt3(wv), wt3(wo)
    p = np.arange(128)[:, None]
    n = np.arange(128)[None, :]
    mask = np.where((p % GRP) == (n % GRP), 0.0, -30000.0).astype(np.float32)
    ident = np.eye(128, dtype=np.float32)

    in_maps = []
    for c in range(N_CORES):
        xs = x2[c * T_CORE:(c + 1) * T_CORE]                  # (2048, 2048)
        xt = np.ascontiguousarray(
            xs.T.reshape(CT, 128, T_CORE).transpose(1, 0, 2))
        in_maps.append({"xt": xt, "wqt": wqt, "wkt": wkt, "wvt": wvt,
                        "wot": wot, "maskd": mask, "identd": ident})
    return in_maps


def kernel(x, wq, wk, wv, wo, inv_freq):
    # inv_freq is unused: RoPE is an identical orthogonal transform on q and k
    # at equal positions, and this attention only contracts same-position q·k,
    # so it cancels exactly (verified vs the fp32 reference: ~6e-6 rel).
    from concourse.bass_utils import run_bass_kernel_spmd

    x = np.asarray(x, dtype=np.float32)
    wq = np.asarray(wq, dtype=np.float32)
    wk = np.asarray(wk, dtype=np.float32)
    wv = np.asarray(wv, dtype=np.float32)
    wo = np.asarray(wo, dtype=np.float32)

    if "nc" not in _CACHED:
        _CACHED["nc"] = _build()
    nc = _CACHED["nc"]

    in_maps = _host_prep(x, wq, wk, wv, wo)
    res = run_bass_kernel_spmd(nc, in_maps, core_ids=list(range(N_CORES)))

    out = np.empty((N_CORES * T_CORE, HIDDEN), dtype=np.float32)
    for c in range(N_CORES):
        ot = res.results[c]["ot"]                              # (128, 16, 2048)
        out[c * T_CORE:(c + 1) * T_CORE] = (
            ot.transpose(2, 1, 0).reshape(T_CORE, HIDDEN))
    return out.reshape(x.shape[0], x.shape[1], HIDDEN)


# revision 16
# speedup vs baseline: 15192.5129x; 15192.5129x over previous
"""Trainium2 Bass kernel for nn_LlamaAttention_6588479832091.

Math notes:
  - The reference attention contracts q and k at the SAME sequence position
    (scores = einsum('bshd,bstd->bsht', q, k)), and RoPE applies the same
    orthogonal transform to q and k at equal positions, so RoPE cancels
    exactly: (P R q)·(P R k) = q·k.  v and the output path never see RoPE.
    The kernel therefore computes: q/k/v projections, per-token 16x16
    cross-head softmax attention, and the output projection.
  - Sharding: data-parallel over the 16384 tokens -> 2048 tokens per core,
    weights replicated.  No collectives.
  - Projections run as float32r matmuls (full PE speed, ~1.4e-4 rel err);
    the small attention matmuls run in plain fp32.

Layouts (host-prepared, all "partition-first" 3D):
  xt   [128, 16, 2048]  xt[p, ct, t] = x_shard[t, ct*128+p]           (f32r)
  wqt  [128, 16, 2048]  wqt[p, ct, m] = wq[m, ct*128+p] / sqrt(128)   (f32r)
  wkt, wvt: same layout as wqt (wk, wv unscaled)                      (f32r)
  wot  [128, 16, 2048]  wot[p, mt, r] = wo[r, mt*128+p]               (f32r)
  mask [128, 128]       0 where p%8 == n%8 else -30000                (f32)
  ident[128, 128]       identity                                      (f32)
  ot   [128, 16, 2048]  ot[p, rt, t] = out_shard[t, rt*128+p]         (f32, output)
"""
import sys

for _p in ("/opt/trn_rl_repo", "/root/.axon_site/_ro/trn_rl_repo"):
    if _p not in sys.path:
        sys.path.insert(0, _p)

import numpy as np

T_CORE = 2048      # tokens per core
N_CORES = 8
H = 16             # heads
HD = 128           # head dim
HIDDEN = 2048
CT = HIDDEN // 128  # 16 contraction tiles
TCH = 512          # token chunk for N=512 matmuls
SUB = 256          # attention sub-chunk tokens
GRP = 8            # tokens per attention group

_CACHED = {}


def _build(phases="PAO"):
    import concourse.mybir as mybir
    import concourse.tile as tile
    import concourse.bacc as bacc

    f32 = mybir.dt.float32
    f32r = mybir.dt.float32r

    nc = bacc.Bacc("TRN2", target_bir_lowering=False, debug=False)

    xt = nc.declare_dram_parameter("xt", [128, CT, T_CORE], f32r, isOutput=False)
    wqt = nc.declare_dram_parameter("wqt", [128, CT, HIDDEN], f32r, isOutput=False)
    wkt = nc.declare_dram_parameter("wkt", [128, CT, HIDDEN], f32r, isOutput=False)
    wvt = nc.declare_dram_parameter("wvt", [128, CT, HIDDEN], f32r, isOutput=False)
    wot = nc.declare_dram_parameter("wot", [128, CT, HIDDEN], f32r, isOutput=False)
    maskd = nc.declare_dram_parameter("maskd", [128, 512], f32, isOutput=False)
    identd = nc.declare_dram_parameter("identd", [128, 128], f32, isOutput=False)
    ot = nc.declare_dram_parameter("ot", [128, CT, T_CORE], f32, isOutput=True)

    with tile.TileContext(nc) as tc:
        with tc.tile_pool(name="dram", bufs=1, space="DRAM") as dram:
            qT = dram.tile([128, H, T_CORE], f32, name="qT")
            kT = dram.tile([128, H, T_CORE], f32, name="kT")
            vT = dram.tile([128, H, T_CORE], f32, name="vT")
            aT = dram.tile([128, H, T_CORE], f32, name="aT")

            # ---------------- Phase P: q/k/v projections (f32r) -------------
            if "P" in phases:
             with tc.tile_pool(name="p_x", bufs=1) as p_x, \
                 tc.tile_pool(name="p_w", bufs=3) as p_w, \
                 tc.tile_pool(name="p_ev", bufs=4) as p_ev, \
                 tc.tile_pool(name="p_ps", bufs=4, space="PSUM") as p_ps:
                x_sb = p_x.tile([128, CT, T_CORE], f32r)
                for ct in range(CT):
                    nc.gpsimd.dma_start(x_sb[:, ct, :], xt[:, ct, :])
                for wsrc, qdst in ((wqt, qT), (wkt, kT), (wvt, vT)):
                    for mt in range(H):
                        wslab = p_w.tile([128, CT, 128], f32r, tag="wslab")
                        nc.sync.dma_start(wslab[:], wsrc[:, :, mt * 128:(mt + 1) * 128])
                        for tch in range(T_CORE // TCH):
                            psum = p_ps.tile([128, TCH], f32, tag="pp")
                            for kt in range(CT):
                                nc.tensor.matmul(
                                    psum[:],
                                    wslab[:, kt, :],
                                    x_sb[:, kt, tch * TCH:(tch + 1) * TCH],
                                    start=(kt == 0), stop=(kt == CT - 1))
                            ev = p_ev.tile([128, TCH], f32, tag="ev")
                            nc.vector.tensor_copy(ev[:], psum[:])
                            nc.sync.dma_start(
                                qdst[:, mt, tch * TCH:(tch + 1) * TCH], ev[:])

            # ---------------- Phase A: cross-head attention ------------------
            # 4 groups (32 tokens) per "macro": wide DVE/ACT ops, software-
            # skewed emission so the in-order PE stream never stalls on the
            # current macro's exp.
            if "A" in phases:
             with tc.tile_pool(name="a_io", bufs=1) as a_io, \
                 tc.tile_pool(name="a_wk", bufs=3) as a_wk, \
                 tc.tile_pool(name="a_ps", bufs=1, space="PSUM") as a_ps:
                mask_sb = a_io.tile([128, 512], f32, tag="mask")
                ident_sb = a_io.tile([128, 128], f32, tag="ident")
                ones_sb = a_io.tile([128, 1], f32, tag="ones")
                nc.sync.dma_start(mask_sb[:], maskd[:])
                nc.sync.dma_start(ident_sb[:], identd[:])
                nc.gpsimd.memset(ones_sb[:], 1.0)
                NG = SUB // GRP          # groups per sub-chunk
                NM = NG // 4             # macros per sub-chunk
                MAC = 4 * GRP            # tokens per macro

                def stage1(st, m):
                    """MM1 x4 + mask + exp for macro m."""
                    ps_s = a_ps.tile([128, 512], f32, tag="s", bufs=2)
                    for i in range(4):
                        g = 4 * m + i
                        nc.tensor.matmul(ps_s[:, i * 128:(i + 1) * 128],
                                         st["k"][:, g, :], st["q"][:, g, :],
                                         start=True, stop=True)
                    nc.vector.tensor_add(ps_s[:], ps_s[:], mask_sb[:])
                    wt = a_wk.tile([128, 512], f32, tag="wt_sb", bufs=3)
                    nc.scalar.activation(wt[:], ps_s[:],
                                         mybir.ActivationFunctionType.Exp)
                    st[("wt", m)] = wt

                def stage2(st, m):
                    """Z + rz + V-transpose + MM2 + normalize + aT for macro m."""
                    wt = st.pop(("wt", m))
                    ps_z = a_ps.tile([128, 4], f32, tag="z", bufs=1)
                    for i in range(4):
                        nc.tensor.matmul(ps_z[:, i:i + 1],
                                         wt[:, i * 128:(i + 1) * 128], ones_sb[:],
                                         start=True, stop=True)
                    rz = a_wk.tile([128, 4], f32, tag="rz", bufs=3)
                    nc.vector.reciprocal(rz[:], ps_z[:])
                    ps_v = a_ps.tile([128, 512], f32, tag="vp", bufs=1)
                    for i in range(4):
                        g = 4 * m + i
                        nc.tensor.transpose(ps_v[:, i * 128:(i + 1) * 128],
                                            st["v"][:, g, :], ident_sb[:])
                    vp = a_wk.tile([128, 512], f32, tag="vp_sb", bufs=3)
                    nc.vector.tensor_copy(vp[:], ps_v[:])
                    ps_at = a_ps.tile([128, 512], f32, tag="attn", bufs=2)
                    for i in range(4):
                        nc.tensor.matmul(ps_at[:, i * 128:(i + 1) * 128],
                                         wt[:, i * 128:(i + 1) * 128],
                                         vp[:, i * 128:(i + 1) * 128],
                                         start=True, stop=True)
                    an = a_wk.tile([128, 512], f32, tag="an_sb", bufs=3)
                    nc.vector.tensor_mul(
                        an[:].rearrange("p (g c) -> p g c", g=4),
                        ps_at[:].rearrange("p (g c) -> p g c", g=4),
                        rz[:].broadcast_to((128, 4, 128)))
                    ps_aT = a_ps.tile([128, 512], f32, tag="aTp", bufs=2)
                    for i in range(4):
                        nc.tensor.transpose(ps_aT[:, i * 128:(i + 1) * 128],
                                            an[:, i * 128:(i + 1) * 128], ident_sb[:])
                    nc.scalar.copy(
                        st["at"][:, :, m * MAC:(m + 1) * MAC].rearrange(
                            "p h (g ti) -> p g h ti", g=4),
                        ps_aT[:].rearrange("p (g h ti) -> p g h ti", g=4, h=H))

                for sub in range(T_CORE // SUB):
                    sl = slice(sub * SUB, (sub + 1) * SUB)
                    st = {}
                    st["q"] = a_io.tile([128, NG, 128], f32, tag="q", bufs=2, name="qpk")
                    st["k"] = a_io.tile([128, NG, 128], f32, tag="k", bufs=2, name="kpk")
                    st["v"] = a_io.tile([128, NG, 128], f32, tag="v", bufs=2, name="vpk")
                    st["at"] = a_io.tile([128, H, SUB], f32, tag="at", bufs=2, name="atsb")
                    # contiguous loads, then on-chip (h,t)->(t,h) repack
                    # (scattered per-head DMAs would swamp the DMA engines)
                    for nm, pk, srcd, eng in (("q", st["q"], qT, nc.vector),
                                              ("k", st["k"], kT, nc.scalar),
                                              ("v", st["v"], vT, nc.scalar)):
                        stg = a_io.tile([128, H, SUB], f32, tag=f"stg_{nm}",
                                        bufs=1, name=f"stg{nm}")
                        nc.sync.dma_start(stg[:], srcd[:, :, sl])
                        dst = pk[:].rearrange("p g (h tj) -> p g h tj", tj=GRP)
                        srcv = stg[:].rearrange("p h (g tj) -> p g h tj", tj=GRP)
                        if eng is nc.vector:
                            nc.vector.tensor_copy(dst, srcv)
                        else:
                            nc.scalar.copy(dst, srcv)
                    for m in range(NM + 1):
                        if m < NM:
                            stage1(st, m)
                        if m >= 1:
                            stage2(st, m - 1)
                    nc.gpsimd.dma_start(aT[:, :, sl], st["at"][:])

            # ---------------- Phase O: output projection (f32r) --------------
            if "O" in phases:
             with tc.tile_pool(name="o_w", bufs=1) as o_w, \
                 tc.tile_pool(name="o_a", bufs=2) as o_a, \
                 tc.tile_pool(name="o_ev", bufs=4) as o_ev, \
                 tc.tile_pool(name="o_ps", bufs=4, space="PSUM") as o_ps:
                wo_sb = o_w.tile([128, CT, CT, 128], f32r)  # [p, rt, kt, r]
                for tch in range(T_CORE // TCH):
                    a_sb = o_a.tile([128, H, TCH], f32r, tag="a")
                    nc.gpsimd.dma_start(
                        a_sb[:], aT[:, :, tch * TCH:(tch + 1) * TCH])
                    for rt in range(CT):
                        if tch == 0:
                            nc.sync.dma_start(wo_sb[:, rt, :, :],
                                              wot[:, :, rt * 128:(rt + 1) * 128])
                        psum = o_ps.tile([128, TCH], f32, tag="po")
                        for kt in range(CT):
                            nc.tensor.matmul(
                                psum[:],
                                wo_sb[:, rt, kt, :],
                                a_sb[:, kt, :],
                                start=(kt == 0), stop=(kt == CT - 1))
                        ev = o_ev.tile([128, TCH], f32, tag="oev")
                        nc.vector.tensor_copy(ev[:], psum[:])
                        nc.sync.dma_start(
                            ot[:, rt, tch * TCH:(tch + 1) * TCH], ev[:])
    nc.compile()
    return nc


def _host_prep(x, wq, wk, wv, wo):
    """Build per-core input maps (layout transforms only)."""
    x2 = np.ascontiguousarray(x.reshape(-1, HIDDEN))          # (16384, 2048)
    wqs = (wq / np.sqrt(np.float32(HD))).astype(np.float32)

    def wt3(w):   # (m, c) weight -> [128, CT, HIDDEN] with w.T tiled on c
        wt = np.ascontiguousarray(w.T)                        # (c, m)
        return np.ascontiguousarray(
            wt.reshape(CT, 128, HIDDEN).transpose(1, 0, 2))

    wqt, wkt, wvt, wot = wt3(wqs), wt3(wk), wt3(wv), wt3(wo)
    p = np.arange(128)[:, None]
    n = np.arange(128)[None, :]
    mask = np.where((p % GRP) == (n % GRP), 0.0, -30000.0).astype(np.float32)
    mask = np.tile(mask, (1, 4))
    ident = np.eye(128, dtype=np.float32)

    in_maps = []
    for c in range(N_CORES):
        xs = x2[c * T_CORE:(c + 1) * T_CORE]                  # (2048, 2048)
        xt = np.ascontiguousarray(
            xs.T.reshape(CT, 128, T_CORE).transpose(1, 0, 2))
        in_maps.append({"xt": xt, "wqt": wqt, "wkt": wkt, "wvt": wvt,
                        "wot": wot, "maskd": mask, "identd": ident})
    return in_maps


def kernel(x, wq, wk, wv, wo, inv_freq):
    # inv_freq is unused: RoPE is an identical orthogonal transform on q and k
    # at equal positions, and this attention only contracts same-position q·k,
    # so it cancels exactly (verified vs the fp32 reference: ~6e-6 rel).
    from concourse.bass_utils import run_bass_kernel_spmd

    x = np.asarray(x, dtype=np.float32)
    wq = np.asarray(wq, dtype=np.float32)
    wk = np.asarray(wk, dtype=np.float32)
    wv = np.asarray(wv, dtype=np.float32)
    wo = np.asarray(wo, dtype=np.float32)

    if "nc" not in _CACHED:
        _CACHED["nc"] = _build()
    nc = _CACHED["nc"]

    in_maps = _host_prep(x, wq, wk, wv, wo)
    res = run_bass_kernel_spmd(nc, in_maps, core_ids=list(range(N_CORES)))

    out = np.empty((N_CORES * T_CORE, HIDDEN), dtype=np.float32)
    for c in range(N_CORES):
        ot = res.results[c]["ot"]                              # (128, 16, 2048)
        out[c * T_CORE:(c + 1) * T_CORE] = (
            ot.transpose(2, 1, 0).reshape(T_CORE, HIDDEN))
    return out.reshape(x.shape[0], x.shape[1], HIDDEN)


# revision 21
# speedup vs baseline: 15566.7592x; 1.0246x over previous
"""Trainium2 Bass kernel for nn_LlamaAttention_6588479832091.

Math notes:
  - The reference attention contracts q and k at the SAME sequence position
    (scores = einsum('bshd,bstd->bsht', q, k)), and RoPE applies the same
    orthogonal transform to q and k at equal positions, so RoPE cancels
    exactly: (P R q)·(P R k) = q·k.  v and the output path never see RoPE.
    The kernel therefore computes: q/k/v projections, per-token 16x16
    cross-head softmax attention, and the output projection.
  - Sharding: data-parallel over the 16384 tokens -> 2048 tokens per core,
    weights replicated.  No collectives.
  - Projections run as float32r matmuls (full PE speed, ~1.4e-4 rel err);
    the small attention matmuls run in plain fp32.

Layouts (host-prepared, all "partition-first" 3D):
  xt   [128, 16, 2048]  xt[p, ct, t] = x_shard[t, ct*128+p]           (f32r)
  wqt  [128, 16, 2048]  wqt[p, ct, m] = wq[m, ct*128+p] / sqrt(128)   (f32r)
  wkt, wvt: same layout as wqt (wk, wv unscaled)                      (f32r)
  wot  [128, 16, 2048]  wot[p, mt, r] = wo[r, mt*128+p]               (f32r)
  mask [128, 128]       0 where p%8 == n%8 else -30000                (f32)
  ident[128, 128]       identity                                      (f32)
  ot   [128, 16, 2048]  ot[p, rt, t] = out_shard[t, rt*128+p]         (f32, output)
"""
import sys

for _p in ("/opt/trn_rl_repo", "/root/.axon_site/_ro/trn_rl_repo"):
    if _p not in sys.path:
        sys.path.insert(0, _p)

import numpy as np

T_CORE = 2048      # tokens per core
N_CORES = 8
H = 16             # heads
HD = 128           # head dim
HIDDEN = 2048
CT = HIDDEN // 128  # 16 contraction tiles
TCH = 512          # token chunk for N=512 matmuls
SUB = 256          # attention sub-chunk tokens
GRP = 8            # tokens per attention group

_CACHED = {}


def _build(phases="PAO"):
    import concourse.mybir as mybir
    import concourse.tile as tile
    import concourse.bacc as bacc

    f32 = mybir.dt.float32
    f32r = mybir.dt.float32r

    nc = bacc.Bacc("TRN2", target_bir_lowering=False, debug=False)

    xt = nc.declare_dram_parameter("xt", [128, CT, T_CORE], f32r, isOutput=False)
    wqt = nc.declare_dram_parameter("wqt", [128, CT, HIDDEN], f32r, isOutput=False)
    wkt = nc.declare_dram_parameter("wkt", [128, CT, HIDDEN], f32r, isOutput=False)
    wvt = nc.declare_dram_parameter("wvt", [128, CT, HIDDEN], f32r, isOutput=False)
    wot = nc.declare_dram_parameter("wot", [128, CT, HIDDEN], f32r, isOutput=False)
    maskd = nc.declare_dram_parameter("maskd", [128, 512], f32, isOutput=False)
    identd = nc.declare_dram_parameter("identd", [128, 128], f32, isOutput=False)
    ot = nc.declare_dram_parameter("ot", [128, CT, T_CORE], f32, isOutput=True)

    with tile.TileContext(nc) as tc:
        with tc.tile_pool(name="dram", bufs=1, space="DRAM") as dram:
            qT = dram.tile([128, H, T_CORE], f32, name="qT")
            kT = dram.tile([128, H, T_CORE], f32, name="kT")
            vT = dram.tile([128, H, T_CORE], f32, name="vT")

            # ---------------- Phase P: q/k/v projections (f32r) -------------
            if "P" in phases:
             with tc.tile_pool(name="p_x", bufs=1) as p_x, \
                 tc.tile_pool(name="p_w", bufs=3) as p_w, \
                 tc.tile_pool(name="p_ev", bufs=4) as p_ev, \
                 tc.tile_pool(name="p_ps", bufs=4, space="PSUM") as p_ps:
                x_sb = p_x.tile([128, CT, T_CORE], f32r)
                for ct in range(CT):
                    nc.gpsimd.dma_start(x_sb[:, ct, :], xt[:, ct, :])
                for wsrc, qdst in ((wqt, qT), (wkt, kT), (wvt, vT)):
                    for mt in range(H):
                        wslab = p_w.tile([128, CT, 128], f32r, tag="wslab")
                        nc.sync.dma_start(wslab[:], wsrc[:, :, mt * 128:(mt + 1) * 128])
                        for tch in range(T_CORE // TCH):
                            psum = p_ps.tile([128, TCH], f32, tag="pp")
                            for kt in range(CT):
                                nc.tensor.matmul(
                                    psum[:],
                                    wslab[:, kt, :],
                                    x_sb[:, kt, tch * TCH:(tch + 1) * TCH],
                                    start=(kt == 0), stop=(kt == CT - 1))
                            ev = p_ev.tile([128, TCH], f32, tag="ev")
                            nc.vector.tensor_copy(ev[:], psum[:])
                            nc.sync.dma_start(
                                qdst[:, mt, tch * TCH:(tch + 1) * TCH], ev[:])

            # ---------------- Phase A: cross-head attention ------------------
            # 4 groups (32 tokens) per "macro": wide DVE/ACT ops, software-
            # skewed emission so the in-order PE stream never stalls on the
            # current macro's exp.
            if "A" in phases:
             with tc.tile_pool(name="a_io", bufs=1) as a_io, \
                 tc.tile_pool(name="a_wk", bufs=3) as a_wk, \
                 tc.tile_pool(name="a_ps", bufs=1, space="PSUM") as a_ps:
                mask_sb = a_io.tile([128, 512], f32, tag="mask")
                ident_sb = a_io.tile([128, 128], f32, tag="ident")
                ones_sb = a_io.tile([128, 1], f32, tag="ones")
                nc.sync.dma_start(mask_sb[:], maskd[:])
                nc.sync.dma_start(ident_sb[:], identd[:])
                nc.gpsimd.memset(ones_sb[:], 1.0)
                NG = SUB // GRP          # groups per sub-chunk
                NM = NG // 4             # macros per sub-chunk
                MAC = 4 * GRP            # tokens per macro

                def stage1(st, m):
                    """MM1 x4 + mask + exp for macro m."""
                    ps_s = a_ps.tile([128, 512], f32, tag="s", bufs=2)
                    for i in range(4):
                        g = 4 * m + i
                        nc.tensor.matmul(ps_s[:, i * 128:(i + 1) * 128],
                                         st["k"][:, g, :], st["q"][:, g, :],
                                         start=True, stop=True)
                    nc.vector.tensor_add(ps_s[:], ps_s[:], mask_sb[:])
                    wt = a_wk.tile([128, 512], f32, tag="wt_sb", bufs=3)
                    nc.scalar.activation(wt[:], ps_s[:],
                                         mybir.ActivationFunctionType.Exp)
                    st[("wt", m)] = wt

                def stage2(st, m):
                    """Z + rz + V-transpose + MM2 + normalize + aT for macro m."""
                    wt = st.pop(("wt", m))
                    ps_z = a_ps.tile([128, 4], f32, tag="z", bufs=1)
                    for i in range(4):
                        nc.tensor.matmul(ps_z[:, i:i + 1],
                                         wt[:, i * 128:(i + 1) * 128], ones_sb[:],
                                         start=True, stop=True)
                    rz = a_wk.tile([128, 4], f32, tag="rz", bufs=3)
                    nc.vector.reciprocal(rz[:], ps_z[:])
                    ps_v = a_ps.tile([128, 512], f32, tag="vp", bufs=1)
                    for i in range(4):
                        g = 4 * m + i
                        nc.tensor.transpose(ps_v[:, i * 128:(i + 1) * 128],
                                            st["v"][:, g, :], ident_sb[:])
                    vp = a_wk.tile([128, 512], f32, tag="vp_sb", bufs=3)
                    nc.vector.tensor_copy(vp[:], ps_v[:])
                    ps_at = a_ps.tile([128, 512], f32, tag="attn", bufs=2)
                    for i in range(4):
                        nc.tensor.matmul(ps_at[:, i * 128:(i + 1) * 128],
                                         wt[:, i * 128:(i + 1) * 128],
                                         vp[:, i * 128:(i + 1) * 128],
                                         start=True, stop=True)
                    an = a_wk.tile([128, 512], f32, tag="an_sb", bufs=3)
                    nc.vector.tensor_mul(
                        an[:].rearrange("p (g c) -> p g c", g=4),
                        ps_at[:].rearrange("p (g c) -> p g c", g=4),
                        rz[:].broadcast_to((128, 4, 128)))
                    ps_aT = a_ps.tile([128, 512], f32, tag="aTp", bufs=1)
                    for i in range(4):
                        nc.tensor.transpose(ps_aT[:, i * 128:(i + 1) * 128],
                                            an[:, i * 128:(i + 1) * 128], ident_sb[:])
                    nc.scalar.copy(
                        st["at"][:, :, m * MAC:(m + 1) * MAC].rearrange(
                            "p h (g ti) -> p g h ti", g=4),
                        ps_aT[:].rearrange("p (g h ti) -> p g h ti", g=4, h=H))

                prev_at = [None]
                for sub in range(T_CORE // SUB):
                    sl = slice(sub * SUB, (sub + 1) * SUB)
                    st = {}
                    st["q"] = a_io.tile([128, NG, 128], f32, tag="q", bufs=1, name="qpk")
                    st["k"] = a_io.tile([128, NG, 128], f32, tag="k", bufs=1, name="kpk")
                    st["v"] = a_io.tile([128, NG, 128], f32, tag="v", bufs=1, name="vpk")
                    st["at"] = a_io.tile([128, H, SUB], f32r, tag="at", bufs=3, name="atsb")
                    # contiguous loads, then on-chip (h,t)->(t,h) repack
                    # (scattered per-head DMAs would swamp the DMA engines)
                    for nm, pk, srcd, eng in (("q", st["q"], qT, nc.vector),
                                              ("k", st["k"], kT, nc.scalar),
                                              ("v", st["v"], vT, nc.scalar)):
                        stg = a_io.tile([128, H, SUB], f32, tag=f"stg_{nm}",
                                        bufs=1, name=f"stg{nm}")
                        nc.sync.dma_start(stg[:], srcd[:, :, sl])
                        dst = pk[:].rearrange("p g (h tj) -> p g h tj", tj=GRP)
                        srcv = stg[:].rearrange("p h (g tj) -> p g h tj", tj=GRP)
                        if eng is nc.vector:
                            nc.vector.tensor_copy(dst, srcv)
                        else:
                            nc.scalar.copy(dst, srcv)
                    for m in range(NM + 1):
                        if m < NM:
                            stage1(st, m)
                        if m >= 1:
                            stage2(st, m - 1)
                    if sub % 2 == 0:
                        prev_at[0] = st["at"]
                    else:
                        tch = sub // 2
                        at_pair = (prev_at[0], st["at"])
                        for rt in range(CT):
                            woslab = a_io.tile([128, CT, 128], f32r,
                                               tag="woslab", bufs=3, name="woslab")
                            nc.sync.dma_start(
                                woslab[:], wot[:, :, rt * 128:(rt + 1) * 128])
                            po = a_ps.tile([128, TCH], f32, tag="po", bufs=1)
                            for kt in range(CT):
                                # half B relies on per-element has_written:
                                # start=True on half A clears the bank; B's
                                # first matmul overwrites its (cleared) region.
                                nc.tensor.matmul(
                                    po[:, :SUB],
                                    woslab[:, kt, :],
                                    at_pair[0][:, kt, :],
                                    start=(kt == 0), stop=False)
                                nc.tensor.matmul(
                                    po[:, SUB:],
                                    woslab[:, kt, :],
                                    at_pair[1][:, kt, :],
                                    start=False, stop=(kt == CT - 1))
                            oev = a_io.tile([128, TCH], f32, tag="oev",
                                            bufs=3, name="oev")
                            nc.vector.tensor_copy(oev[:], po[:])
                            nc.sync.dma_start(
                                ot[:, rt, tch * TCH:(tch + 1) * TCH], oev[:])

    nc.compile()
    return nc


def _host_prep(x, wq, wk, wv, wo):
    """Build per-core input maps (layout transforms only)."""
    x2 = np.ascontiguousarray(x.reshape(-1, HIDDEN))          # (16384, 2048)
    wqs = (wq / np.sqrt(np.float32(HD))).astype(np.float32)

    def wt3(w):   # (m, c) weight -> [128, CT, HIDDEN] with w.T tiled on c
        wt = np.ascontiguousarray(w.T)                        # (c, m)
        return np.ascontiguousarray(
            wt.reshape(CT, 128, HIDDEN).transpose(1, 0, 2))

    wqt, wkt, wvt, wot = wt3(wqs), wt3(wk), wt3(wv), wt3(wo)
    p = np.arange(128)[:, None]
    n = np.arange(128)[None, :]
    mask = np.where((p % GRP) == (n % GRP), 0.0, -30000.0).astype(np.float32)
    mask = np.tile(mask, (1, 4))
    ident = np.eye(128, dtype=np.float32)

    in_maps = []
    for c in range(N_CORES):
        xs = x2[c * T_CORE:(c + 1) * T_CORE]                  # (2048, 2048)
        xt = np.ascontiguousarray(
            xs.T.reshape(CT, 128, T_CORE).transpose(1, 0, 2))
        in_maps.append({"xt": xt, "wqt": wqt, "wkt": wkt, "wvt": wvt,
                        "wot": wot, "maskd": mask, "identd": ident})
    return in_maps


def kernel(x, wq, wk, wv, wo, inv_freq):
    # inv_freq is unused: RoPE is an identical orthogonal transform on q and k
    # at equal positions, and this attention only contracts same-position q·k,
    # so it cancels exactly (verified vs the fp32 reference: ~6e-6 rel).
    from concourse.bass_utils import run_bass_kernel_spmd

    x = np.asarray(x, dtype=np.float32)
    wq = np.asarray(wq, dtype=np.float32)
    wk = np.asarray(wk, dtype=np.float32)
    wv = np.asarray(wv, dtype=np.float32)
    wo = np.asarray(wo, dtype=np.float32)

    if "nc" not in _CACHED:
        _CACHED["nc"] = _build()
    nc = _CACHED["nc"]

    in_maps = _host_prep(x, wq, wk, wv, wo)
    res = run_bass_kernel_spmd(nc, in_maps, core_ids=list(range(N_CORES)))

    out = np.empty((N_CORES * T_CORE, HIDDEN), dtype=np.float32)
    for c in range(N_CORES):
        ot = res.results[c]["ot"]                              # (128, 16, 2048)
        out[c * T_CORE:(c + 1) * T_CORE] = (
            ot.transpose(2, 1, 0).reshape(T_CORE, HIDDEN))
    return out.reshape(x.shape[0], x.shape[1], HIDDEN)


# revision 30
# speedup vs baseline: 15665.8006x; 1.0064x over previous
"""Trainium2 Bass kernel for nn_LlamaAttention_6588479832091.

Math notes:
  - The reference attention contracts q and k at the SAME sequence position
    (scores = einsum('bshd,bstd->bsht', q, k)), and RoPE applies the same
    orthogonal transform to q and k at equal positions, so RoPE cancels
    exactly: (P R q)·(P R k) = q·k.  v and the output path never see RoPE.
    The kernel therefore computes: q/k/v projections, per-token 16x16
    cross-head softmax attention, and the output projection.
  - Sharding: data-parallel over the 16384 tokens -> 2048 tokens per core,
    weights replicated.  No collectives.
  - Projections run as float32r matmuls (full PE speed, ~1.4e-4 rel err);
    the small attention matmuls run in plain fp32.

Layouts (host-prepared, all "partition-first" 3D):
  xt   [128, 16, 2048]  xt[p, ct, t] = x_shard[t, ct*128+p]           (f32r)
  wqt  [128, 16, 2048]  wqt[p, ct, m] = wq[m, ct*128+p] / sqrt(128)   (f32r)
  wkt, wvt: same layout as wqt (wk, wv unscaled)                      (f32r)
  wot  [128, 16, 2048]  wot[p, mt, r] = wo[r, mt*128+p]               (f32r)
  mask [128, 128]       0 where p%8 == n%8 else -30000                (f32)
  ident[128, 128]       identity                                      (f32)
  ot   [128, 16, 2048]  ot[p, rt, t] = out_shard[t, rt*128+p]         (f32, output)
"""
import sys

for _p in ("/opt/trn_rl_repo", "/root/.axon_site/_ro/trn_rl_repo"):
    if _p not in sys.path:
        sys.path.insert(0, _p)

import numpy as np

T_CORE = 2048      # tokens per core
N_CORES = 8
H = 16             # heads
HD = 128           # head dim
HIDDEN = 2048
CT = HIDDEN // 128  # 16 contraction tiles
TCH = 512          # token chunk for N=512 matmuls
SUB = 256          # attention sub-chunk tokens
GRP = 8            # tokens per attention group

_CACHED = {}


def _build(phases="PAO"):
    import concourse.mybir as mybir
    import concourse.tile as tile
    import concourse.bacc as bacc

    f32 = mybir.dt.float32
    f32r = mybir.dt.float32r

    nc = bacc.Bacc("TRN2", target_bir_lowering=False, debug=False)

    xt = nc.declare_dram_parameter("xt", [128, CT, T_CORE], f32r, isOutput=False)
    wqt = nc.declare_dram_parameter("wqt", [128, CT, HIDDEN], f32r, isOutput=False)
    wkt = nc.declare_dram_parameter("wkt", [128, CT, HIDDEN], f32r, isOutput=False)
    wvt = nc.declare_dram_parameter("wvt", [128, CT, HIDDEN], f32r, isOutput=False)
    wot = nc.declare_dram_parameter("wot", [128, CT, HIDDEN], f32r, isOutput=False)
    maskd = nc.declare_dram_parameter("maskd", [128, 512], f32, isOutput=False)
    identd = nc.declare_dram_parameter("identd", [128, 128], f32, isOutput=False)
    ot = nc.declare_dram_parameter("ot", [128, CT, T_CORE], f32, isOutput=True)

    with tile.TileContext(nc) as tc:
        with tc.tile_pool(name="dram", bufs=1, space="DRAM") as dram:
            NTCH = T_CORE // TCH
            qT = [dram.tile([128, H, TCH], f32, name=f"qT{i}") for i in range(NTCH)]
            kT = [dram.tile([128, H, TCH], f32, name=f"kT{i}") for i in range(NTCH)]
            vT = [dram.tile([128, H, TCH], f32, name=f"vT{i}") for i in range(NTCH)]

            # ---------------- Phase P: q/k/v projections (f32r) -------------
            if "P" in phases:
             with tc.tile_pool(name="p_x", bufs=1) as p_x, \
                 tc.tile_pool(name="p_w", bufs=4) as p_w, \
                 tc.tile_pool(name="p_ev", bufs=6) as p_ev, \
                 tc.tile_pool(name="p_ps", bufs=6, space="PSUM") as p_ps:
                x_sb = p_x.tile([128, CT, T_CORE], f32r)
                for ct in range(CT):
                    nc.gpsimd.dma_start(x_sb[:, ct, :], xt[:, ct, :])
                for wsrc, qdst in ((wqt, qT), (wkt, kT), (wvt, vT)):
                    for mt in range(H):
                        wslab = p_w.tile([128, CT, 128], f32r, tag="wslab")
                        nc.sync.dma_start(wslab[:], wsrc[:, :, mt * 128:(mt + 1) * 128])
                        for tch in range(T_CORE // TCH):
                            psum = p_ps.tile([128, TCH], f32, tag="pp")
                            for kt in range(CT):
                                nc.tensor.matmul(
                                    psum[:],
                                    wslab[:, kt, :],
                                    x_sb[:, kt, tch * TCH:(tch + 1) * TCH],
                                    start=(kt == 0), stop=(kt == CT - 1))
                            ev = p_ev.tile([128, TCH], f32, tag="ev")
                            nc.vector.tensor_copy(ev[:], psum[:])
                            nc.sync.dma_start(qdst[tch][:, mt, :], ev[:])

            # ---------------- Phase A: cross-head attention ------------------
            # 4 groups (32 tokens) per "macro": wide DVE/ACT ops, software-
            # skewed emission so the in-order PE stream never stalls on the
            # current macro's exp.
            if "A" in phases:
             with tc.tile_pool(name="a_io", bufs=1) as a_io, \
                 tc.tile_pool(name="a_wk", bufs=3) as a_wk, \
                 tc.tile_pool(name="a_ps", bufs=1, space="PSUM") as a_ps:
                mask_sb = a_io.tile([128, 512], f32, tag="mask")
                ident_sb = a_io.tile([128, 128], f32, tag="ident")
                ones_sb = a_io.tile([128, 1], f32, tag="ones")
                nc.sync.dma_start(mask_sb[:], maskd[:])
                nc.sync.dma_start(ident_sb[:], identd[:])
                nc.gpsimd.memset(ones_sb[:], 1.0)
                NG = SUB // GRP          # groups per sub-chunk
                NM = NG // 4             # macros per sub-chunk
                MAC = 4 * GRP            # tokens per macro

                def stage1(st, m):
                    """MM1 x4 + mask + exp for macro m."""
                    ps_s = a_ps.tile([128, 512], f32, tag="s", bufs=2)
                    for i in range(4):
                        g = 4 * m + i
                        nc.tensor.matmul(ps_s[:, i * 128:(i + 1) * 128],
                                         st["k"][:, g, :], st["q"][:, g, :],
                                         start=True, stop=True)
                    nc.vector.tensor_add(ps_s[:], ps_s[:], mask_sb[:])
                    wt = a_wk.tile([128, 512], f32, tag="wt_sb", bufs=4)
                    nc.scalar.activation(wt[:], ps_s[:],
                                         mybir.ActivationFunctionType.Exp)
                    st[("wt", m)] = wt

                def stage2(st, m):
                    """Z + rz + V-transpose + MM2 + normalize + aT for macro m."""
                    wt = st.pop(("wt", m))
                    ps_z = a_ps.tile([128, 4], f32, tag="z", bufs=1)
                    for i in range(4):
                        nc.tensor.matmul(ps_z[:, i:i + 1],
                                         wt[:, i * 128:(i + 1) * 128], ones_sb[:],
                                         start=True, stop=True)
                    rz = a_wk.tile([128, 4], f32, tag="rz", bufs=3)
                    nc.vector.reciprocal(rz[:], ps_z[:])
                    ps_v = a_ps.tile([128, 512], f32, tag="vp", bufs=1)
                    for i in range(4):
                        g = 4 * m + i
                        nc.tensor.transpose(ps_v[:, i * 128:(i + 1) * 128],
                                            st["v"][:, g, :], ident_sb[:])
                    vp = a_wk.tile([128, 512], f32, tag="vp_sb", bufs=4)
                    nc.vector.tensor_copy(vp[:], ps_v[:])
                    ps_at = a_ps.tile([128, 512], f32, tag="attn", bufs=3)
                    for i in range(4):
                        nc.tensor.matmul(ps_at[:, i * 128:(i + 1) * 128],
                                         wt[:, i * 128:(i + 1) * 128],
                                         vp[:, i * 128:(i + 1) * 128],
                                         start=True, stop=True)
                    an = a_wk.tile([128, 512], f32, tag="an_sb", bufs=4)
                    nc.vector.tensor_mul(
                        an[:].rearrange("p (g c) -> p g c", g=4),
                        ps_at[:].rearrange("p (g c) -> p g c", g=4),
                        rz[:].broadcast_to((128, 4, 128)))
                    ps_aT = a_ps.tile([128, 512], f32, tag="aTp", bufs=1)
                    for i in range(4):
                        nc.tensor.transpose(ps_aT[:, i * 128:(i + 1) * 128],
                                            an[:, i * 128:(i + 1) * 128], ident_sb[:])
                    nc.scalar.copy(
                        st["at"][:, :, m * MAC:(m + 1) * MAC].rearrange(
                            "p h (g ti) -> p g h ti", g=4),
                        ps_aT[:].rearrange("p (g h ti) -> p g h ti", g=4, h=H))

                prev_at = [None]
                for sub in range(T_CORE // SUB):
                    sl = slice(sub * SUB, (sub + 1) * SUB)
                    st = {}
                    st["q"] = a_io.tile([128, NG, 128], f32, tag="q", bufs=1, name="qpk")
                    st["k"] = a_io.tile([128, NG, 128], f32, tag="k", bufs=1, name="kpk")
                    st["v"] = a_io.tile([128, NG, 128], f32, tag="v", bufs=1, name="vpk")
                    st["at"] = a_io.tile([128, H, SUB], f32r, tag="at", bufs=3, name="atsb")
                    # contiguous loads, then on-chip (h,t)->(t,h) repack
                    # (scattered per-head DMAs would swamp the DMA engines)
                    tch_i, half = sub // 2, (sub % 2) * SUB
                    for nm, pk, srcd, eng in (("q", st["q"], qT, nc.vector),
                                              ("k", st["k"], kT, nc.scalar),
                                              ("v", st["v"], vT, nc.scalar)):
                        stg = a_io.tile([128, H, SUB], f32, tag=f"stg_{nm}",
                                        bufs=1, name=f"stg{nm}")
                        nc.sync.dma_start(stg[:], srcd[tch_i][:, :, half:half + SUB])
                        dst = pk[:].rearrange("p g (h tj) -> p g h tj", tj=GRP)
                        srcv = stg[:].rearrange("p h (g tj) -> p g h tj", tj=GRP)
                        if eng is nc.vector:
                            nc.vector.tensor_copy(dst, srcv)
                        else:
                            nc.scalar.copy(dst, srcv)
                    for m in range(NM + 1):
                        if m < NM:
                            stage1(st, m)
                        if m >= 1:
                            stage2(st, m - 1)
                    if sub % 2 == 0:
                        prev_at[0] = st["at"]
                    else:
                        tch = sub // 2
                        at_pair = (prev_at[0], st["at"])
                        for rt in range(CT):
                            woslab = a_io.tile([128, CT, 128], f32r,
                                               tag="woslab", bufs=3, name="woslab")
                            nc.sync.dma_start(
                                woslab[:], wot[:, :, rt * 128:(rt + 1) * 128])
                            po = a_ps.tile([128, TCH], f32, tag="attn", bufs=3)
                            for kt in range(CT):
                                # half B relies on per-element has_written:
                                # start=True on half A clears the bank; B's
                                # first matmul overwrites its (cleared) region.
                                nc.tensor.matmul(
                                    po[:, :SUB],
                                    woslab[:, kt, :],
                                    at_pair[0][:, kt, :],
                                    start=(kt == 0), stop=False)
                                nc.tensor.matmul(
                                    po[:, SUB:],
                                    woslab[:, kt, :],
                                    at_pair[1][:, kt, :],
                                    start=False, stop=(kt == CT - 1))
                            oev = a_io.tile([128, TCH], f32, tag="oev",
                                            bufs=4, name="oev")
                            nc.vector.tensor_copy(oev[:], po[:])
                            nc.sync.dma_start(
                                ot[:, rt, tch * TCH:(tch + 1) * TCH], oev[:])

    nc.compile()
    return nc


def _host_prep(x, wq, wk, wv, wo):
    """Build per-core input maps (layout transforms only)."""
    x2 = np.ascontiguousarray(x.reshape(-1, HIDDEN))          # (16384, 2048)
    wqs = (wq / np.sqrt(np.float32(HD))).astype(np.float32)

    def wt3(w):   # (m, c) weight -> [128, CT, HIDDEN] with w.T tiled on c
        wt = np.ascontiguousarray(w.T)                        # (c, m)
        return np.ascontiguousarray(
            wt.reshape(CT, 128, HIDDEN).transpose(1, 0, 2))

    wqt, wkt, wvt, wot = wt3(wqs), wt3(wk), wt3(wv), wt3(wo)
    p = np.arange(128)[:, None]
    n = np.arange(128)[None, :]
    mask = np.where((p % GRP) == (n % GRP), 0.0, -30000.0).astype(np.float32)
    mask = np.tile(mask, (1, 4))
    ident = np.eye(128, dtype=np.float32)

    in_maps = []
    for c in range(N_CORES):
        xs = x2[c * T_CORE:(c + 1) * T_CORE]                  # (2048, 2048)
        xt = np.ascontiguousarray(
            xs.T.reshape(CT, 128, T_CORE).transpose(1, 0, 2))
        in_maps.append({"xt": xt, "wqt": wqt, "wkt": wkt, "wvt": wvt,
                        "wot": wot, "maskd": mask, "identd": ident})
    return in_maps


def kernel(x, wq, wk, wv, wo, inv_freq):
    # inv_freq is unused: RoPE is an identical orthogonal transform on q and k
    # at equal positions, and this attention only contracts same-position q·k,
    # so it cancels exactly (verified vs the fp32 reference: ~6e-6 rel).
    from concourse.bass_utils import run_bass_kernel_spmd

    x = np.asarray(x, dtype=np.float32)
    wq = np.asarray(wq, dtype=np.float32)
    wk = np.asarray(wk, dtype=np.float32)
    wv = np.asarray(wv, dtype=np.float32)
    wo = np.asarray(wo, dtype=np.float32)

    if "nc" not in _CACHED:
        _CACHED["nc"] = _build()
    nc = _CACHED["nc"]

    in_maps = _host_prep(x, wq, wk, wv, wo)
    res = run_bass_kernel_spmd(nc, in_maps, core_ids=list(range(N_CORES)))

    out = np.empty((N_CORES * T_CORE, HIDDEN), dtype=np.float32)
    for c in range(N_CORES):
        ot = res.results[c]["ot"]                              # (128, 16, 2048)
        out[c * T_CORE:(c + 1) * T_CORE] = (
            ot.transpose(2, 1, 0).reshape(T_CORE, HIDDEN))
    return out.reshape(x.shape[0], x.shape[1], HIDDEN)


# revision 37
# speedup vs baseline: 15991.0172x; 1.0208x over previous
"""Trainium2 Bass kernel for nn_LlamaAttention_6588479832091.

Math notes:
  - The reference attention contracts q and k at the SAME sequence position
    (scores = einsum('bshd,bstd->bsht', q, k)), and RoPE applies the same
    orthogonal transform to q and k at equal positions, so RoPE cancels
    exactly: (P R q)·(P R k) = q·k.  v and the output path never see RoPE.
    The kernel therefore computes: q/k/v projections, per-token 16x16
    cross-head softmax attention, and the output projection.
  - Sharding: data-parallel over the 16384 tokens -> 2048 tokens per core,
    weights replicated.  No collectives.
  - Projections run as float32r matmuls (full PE speed, ~1.4e-4 rel err);
    the small attention matmuls run in plain fp32.

Layouts (host-prepared, all "partition-first" 3D):
  xt   [128, 16, 2048]  xt[p, ct, t] = x_shard[t, ct*128+p]           (f32r)
  wqt  [128, 16, 2048]  wqt[p, ct, m] = wq[m, ct*128+p] / sqrt(128)   (f32r)
  wkt, wvt: same layout as wqt (wk, wv unscaled)                      (f32r)
  wot  [128, 16, 2048]  wot[p, mt, r] = wo[r, mt*128+p]               (f32r)
  mask [128, 128]       0 where p%8 == n%8 else -30000                (f32)
  ident[128, 128]       identity                                      (f32)
  ot   [128, 16, 2048]  ot[p, rt, t] = out_shard[t, rt*128+p]         (f32, output)
"""
import sys

for _p in ("/opt/trn_rl_repo", "/root/.axon_site/_ro/trn_rl_repo"):
    if _p not in sys.path:
        sys.path.insert(0, _p)

import numpy as np

T_CORE = 2048      # tokens per core
N_CORES = 8
H = 16             # heads
HD = 128           # head dim
HIDDEN = 2048
CT = HIDDEN // 128  # 16 contraction tiles
TCH = 512          # token chunk for N=512 matmuls
SUB = 128          # attention sub-chunk tokens
ATW = 256          # attn-output tile width (written by 2 subs)
GRP = 8            # tokens per attention group

_CACHED = {}


def _build(phases="PAO"):
    import concourse.mybir as mybir
    import concourse.tile as tile
    import concourse.bacc as bacc

    f32 = mybir.dt.float32
    f32r = mybir.dt.float32r

    nc = bacc.Bacc("TRN2", target_bir_lowering=False, debug=False)

    xt = nc.declare_dram_parameter("xt", [128, CT, T_CORE], f32r, isOutput=False)
    wqt = nc.declare_dram_parameter("wqt", [128, CT, HIDDEN], f32r, isOutput=False)
    wkt = nc.declare_dram_parameter("wkt", [128, CT, HIDDEN], f32r, isOutput=False)
    wvt = nc.declare_dram_parameter("wvt", [128, CT, HIDDEN], f32r, isOutput=False)
    wot = nc.declare_dram_parameter("wot", [128, CT, HIDDEN], f32r, isOutput=False)
    maskd = nc.declare_dram_parameter("maskd", [128, 512], f32, isOutput=False)
    identd = nc.declare_dram_parameter("identd", [128, 128], f32, isOutput=False)
    ot = nc.declare_dram_parameter("ot", [128, CT, T_CORE], f32, isOutput=True)

    with tile.TileContext(nc) as tc:
        with tc.tile_pool(name="dram", bufs=1, space="DRAM") as dram:
            NTCH = T_CORE // TCH
            qT = [dram.tile([128, H, TCH], f32, name=f"qT{i}") for i in range(NTCH)]
            kT = [dram.tile([128, H, TCH], f32, name=f"kT{i}") for i in range(NTCH)]
            vT = [dram.tile([128, H, TCH], f32, name=f"vT{i}") for i in range(NTCH)]

            # ---------------- Phase P: q/k/v projections (f32r) -------------
            if "P" in phases:
             with tc.tile_pool(name="p_x", bufs=1) as p_x, \
                 tc.tile_pool(name="p_w", bufs=4) as p_w, \
                 tc.tile_pool(name="p_ev", bufs=6) as p_ev, \
                 tc.tile_pool(name="p_ps", bufs=6, space="PSUM") as p_ps:
                x_sb = p_x.tile([128, CT, T_CORE], f32r)
                for ct in range(CT):
                    nc.gpsimd.dma_start(x_sb[:, ct, :], xt[:, ct, :])
                for wsrc, qdst in ((wqt, qT), (wkt, kT), (wvt, vT)):
                    for mt in range(H):
                        wslab = p_w.tile([128, CT, 128], f32r, tag="wslab")
                        nc.sync.dma_start(wslab[:], wsrc[:, :, mt * 128:(mt + 1) * 128])
                        for tch in range(T_CORE // TCH):
                            psum = p_ps.tile([128, TCH], f32, tag="pp")
                            for kt in range(CT):
                                nc.tensor.matmul(
                                    psum[:],
                                    wslab[:, kt, :],
                                    x_sb[:, kt, tch * TCH:(tch + 1) * TCH],
                                    start=(kt == 0), stop=(kt == CT - 1))
                            ev = p_ev.tile([128, TCH], f32, tag="ev")
                            nc.vector.tensor_copy(ev[:], psum[:])
                            nc.sync.dma_start(qdst[tch][:, mt, :], ev[:])

            # ---------------- Phase A: cross-head attention ------------------
            # 4 groups (32 tokens) per "macro": wide DVE/ACT ops, software-
            # skewed emission so the in-order PE stream never stalls on the
            # current macro's exp.
            if "A" in phases:
             with tc.tile_pool(name="a_io", bufs=1) as a_io, \
                 tc.tile_pool(name="a_wk", bufs=3) as a_wk, \
                 tc.tile_pool(name="a_ps", bufs=1, space="PSUM") as a_ps:
                mask_sb = a_io.tile([128, 512], f32, tag="mask")
                ident_sb = a_io.tile([128, 128], f32, tag="ident")
                ones_sb = a_io.tile([128, 1], f32, tag="ones")
                nc.sync.dma_start(mask_sb[:], maskd[:])
                nc.sync.dma_start(ident_sb[:], identd[:])
                nc.gpsimd.memset(ones_sb[:], 1.0)
                NG = SUB // GRP          # groups per sub-chunk
                NM = NG // 4             # macros per sub-chunk
                MAC = 4 * GRP            # tokens per macro

                def stage1(st, m):
                    """MM1 x4 + mask + exp for macro m."""
                    ps_s = a_ps.tile([128, 512], f32, tag="s", bufs=2)
                    for i in range(4):
                        g = 4 * m + i
                        nc.tensor.matmul(ps_s[:, i * 128:(i + 1) * 128],
                                         st["k"][:, g, :], st["q"][:, g, :],
                                         start=True, stop=True)
                    nc.vector.tensor_add(ps_s[:], ps_s[:], mask_sb[:])
                    wt = a_wk.tile([128, 512], f32, tag="wt_sb", bufs=4)
                    nc.scalar.activation(wt[:], ps_s[:],
                                         mybir.ActivationFunctionType.Exp)
                    st[("wt", m)] = wt

                def stage2(st, m):
                    """Z + rz + V-transpose + MM2 + normalize + aT for macro m."""
                    wt = st.pop(("wt", m))
                    ps_z = a_ps.tile([128, 4], f32, tag="z", bufs=1)
                    for i in range(4):
                        nc.tensor.matmul(ps_z[:, i:i + 1],
                                         wt[:, i * 128:(i + 1) * 128], ones_sb[:],
                                         start=True, stop=True)
                    rz = a_wk.tile([128, 4], f32, tag="rz", bufs=3)
                    nc.vector.reciprocal(rz[:], ps_z[:])
                    ps_v = a_ps.tile([128, 512], f32, tag="vp", bufs=1)
                    for i in range(4):
                        g = 4 * m + i
                        nc.tensor.transpose(ps_v[:, i * 128:(i + 1) * 128],
                                            st["v"][:, g, :], ident_sb[:])
                    vp = a_wk.tile([128, 512], f32, tag="vp_sb", bufs=4)
                    nc.vector.tensor_copy(vp[:], ps_v[:])
                    ps_at = a_ps.tile([128, 512], f32, tag="attn", bufs=3)
                    for i in range(4):
                        nc.tensor.matmul(ps_at[:, i * 128:(i + 1) * 128],
                                         wt[:, i * 128:(i + 1) * 128],
                                         vp[:, i * 128:(i + 1) * 128],
                                         start=True, stop=True)
                    an = a_wk.tile([128, 512], f32, tag="an_sb", bufs=4)
                    nc.vector.tensor_mul(
                        an[:].rearrange("p (g c) -> p g c", g=4),
                        ps_at[:].rearrange("p (g c) -> p g c", g=4),
                        rz[:].broadcast_to((128, 4, 128)))
                    ps_aT = a_ps.tile([128, 512], f32, tag="aTp", bufs=1)
                    for i in range(4):
                        nc.tensor.transpose(ps_aT[:, i * 128:(i + 1) * 128],
                                            an[:, i * 128:(i + 1) * 128], ident_sb[:])
                    nc.scalar.copy(
                        st["at"][:, :, m * MAC:(m + 1) * MAC].rearrange(
                            "p h (g ti) -> p g h ti", g=4),
                        ps_aT[:].rearrange("p (g h ti) -> p g h ti", g=4, h=H))

                prev_at = [None, None]
                for sub in range(T_CORE // SUB):
                    sl = slice(sub * SUB, (sub + 1) * SUB)
                    st = {}
                    st["q"] = a_io.tile([128, NG, 128], f32, tag="q", bufs=2, name="qpk")
                    st["k"] = a_io.tile([128, NG, 128], f32, tag="k", bufs=2, name="kpk")
                    st["v"] = a_io.tile([128, NG, 128], f32, tag="v", bufs=2, name="vpk")
                    if sub % 2 == 0:
                        at_full = a_io.tile([128, H, ATW], f32r, tag="at", bufs=3,
                                            name="atsb")
                        prev_at = [prev_at[1], at_full]
                    st["at"] = prev_at[1][:, :, (sub % 2) * SUB:(sub % 2) * SUB + SUB]
                    tch_i = sub * SUB // TCH
                    half = (sub * SUB) % TCH
                    for nm, pk, srcd, eng in (("q", st["q"], qT, nc.vector),
                                              ("k", st["k"], kT, nc.scalar),
                                              ("v", st["v"], vT, nc.scalar)):
                        stg = a_io.tile([128, H, SUB], f32, tag=f"stg_{nm}",
                                        bufs=2, name=f"stg{nm}")
                        nc.sync.dma_start(stg[:], srcd[tch_i][:, :, half:half + SUB])
                        dst = pk[:].rearrange("p g (h tj) -> p g h tj", tj=GRP)
                        srcv = stg[:].rearrange("p h (g tj) -> p g h tj", tj=GRP)
                        if eng is nc.vector:
                            nc.vector.tensor_copy(dst, srcv)
                        else:
                            nc.scalar.copy(dst, srcv)
                    for m in range(NM + 1):
                        if m < NM:
                            stage1(st, m)
                        if m >= 1:
                            stage2(st, m - 1)
                    if sub % 4 == 3:
                        tch = sub // 4
                        at_pair = (prev_at[0], prev_at[1])
                        for rt in range(CT):
                            woslab = a_io.tile([128, CT, 128], f32r,
                                               tag="woslab", bufs=3, name="woslab")
                            nc.sync.dma_start(
                                woslab[:], wot[:, :, rt * 128:(rt + 1) * 128])
                            po = a_ps.tile([128, TCH], f32, tag="attn", bufs=3)
                            for kt in range(CT):
                                nc.tensor.matmul(
                                    po[:, :ATW],
                                    woslab[:, kt, :],
                                    at_pair[0][:, kt, :],
                                    start=(kt == 0), stop=False)
                                nc.tensor.matmul(
                                    po[:, ATW:],
                                    woslab[:, kt, :],
                                    at_pair[1][:, kt, :],
                                    start=False, stop=(kt == CT - 1))
                            oev = a_io.tile([128, TCH], f32, tag="oev",
                                            bufs=4, name="oev")
                            nc.vector.tensor_copy(oev[:], po[:])
                            nc.sync.dma_start(
                                ot[:, rt, tch * TCH:(tch + 1) * TCH], oev[:])
    nc.compile()
    return nc


def _host_prep(x, wq, wk, wv, wo):
    """Build per-core input maps (layout transforms only)."""
    x2 = np.ascontiguousarray(x.reshape(-1, HIDDEN))          # (16384, 2048)
    wqs = (wq / np.sqrt(np.float32(HD))).astype(np.float32)

    def wt3(w):   # (m, c) weight -> [128, CT, HIDDEN] with w.T tiled on c
        wt = np.ascontiguousarray(w.T)                        # (c, m)
        return np.ascontiguousarray(
            wt.reshape(CT, 128, HIDDEN).transpose(1, 0, 2))

    wqt, wkt, wvt, wot = wt3(wqs), wt3(wk), wt3(wv), wt3(wo)
    p = np.arange(128)[:, None]
    n = np.arange(128)[None, :]
    mask = np.where((p % GRP) == (n % GRP), 0.0, -30000.0).astype(np.float32)
    mask = np.tile(mask, (1, 4))
    ident = np.eye(128, dtype=np.float32)

    in_maps = []
    for c in range(N_CORES):
        xs = x2[c * T_CORE:(c + 1) * T_CORE]                  # (2048, 2048)
        xt = np.ascontiguousarray(
            xs.T.reshape(CT, 128, T_CORE).transpose(1, 0, 2))
        in_maps.append({"xt": xt, "wqt": wqt, "wkt": wkt, "wvt": wvt,
                        "wot": wot, "maskd": mask, "identd": ident})
    return in_maps


def kernel(x, wq, wk, wv, wo, inv_freq):
    # inv_freq is unused: RoPE is an identical orthogonal transform on q and k
    # at equal positions, and this attention only contracts same-position q·k,
    # so it cancels exactly (verified vs the fp32 reference: ~6e-6 rel).
    from concourse.bass_utils import run_bass_kernel_spmd

    x = np.asarray(x, dtype=np.float32)
    wq = np.asarray(wq, dtype=np.float32)
    wk = np.asarray(wk, dtype=np.float32)
    wv = np.asarray(wv, dtype=np.float32)
    wo = np.asarray(wo, dtype=np.float32)

    if "nc" not in _CACHED:
        _CACHED["nc"] = _build()
    nc = _CACHED["nc"]

    in_maps = _host_prep(x, wq, wk, wv, wo)
    res = run_bass_kernel_spmd(nc, in_maps, core_ids=list(range(N_CORES)))

    out = np.empty((N_CORES * T_CORE, HIDDEN), dtype=np.float32)
    for c in range(N_CORES):
        ot = res.results[c]["ot"]                              # (128, 16, 2048)
        out[c * T_CORE:(c + 1) * T_CORE] = (
            ot.transpose(2, 1, 0).reshape(T_CORE, HIDDEN))
    return out.reshape(x.shape[0], x.shape[1], HIDDEN)
